# revision 31
# baseline (speedup 1.0000x reference)
"""Bahdanau additive attention on 8 TRN2 NeuronCores (batch-parallel).

Math: scores[b,i,j] = q[b,i].w + k[b,j].w, masked to -1e9 where mask==0,
softmax over j, then @ value.  The query term q[b,i].w is constant along j,
so it cancels in the softmax:

    out[b,i,:] = (sum_j mask[b,i,j] * e[b,j] * value[b,j,:])
               / (sum_j mask[b,i,j] * e[b,j]),      e[b,j] = exp(k[b,j].w)

(no query needed, no [Lq,Lk] softmax).  Per core: one batch.

The host uploads the pre-transposed mask directly as fp8e4 bytes
(0x00 / 0x38 = 0.0 / 1.0), so the mask is a PE-ready stationary operand
straight out of the DMA -- no on-chip conversion at all, and (measured)
fp8 LDWEIGHTS is ~97ns, under the 258-col matmul's 108ns, so the stream
runs at the fp16 rate of ~112ns/MM.  k and v stay fp16: any quantization
of k or v is multiplicative noise on the softmax weights / values, and
the output (a random walk in v) inherits it 1:1 -- fp8 k measured 3e-2
rel err.  The e_j scale rides the moving operand: ev rows [e*v | e | 0]
(258 wide) built per strip by one DVE/ACT op from host-packed [v | 1 | 0]
records.  matmul(psum[i, 0:258] += maskT[j,i] * ev[j,:]) accumulates
over 16 j-strips; col 256 is the softmax denominator.

Supply is the whole game: wave A (i-tiles 0-7, strip-major) consumes
mask+k+v bytes at ~430 GB/s -- over the per-core HBM limit -- so wave A
is DMA-paced and ALL loads go on ONE HWDGE ring (sync) in exact
consumption order; no compute op ever queues ahead of a load dispatch
on that engine.  A tiny head tensor [w | k0 | vx0] starts the stream
~1us earlier than a monolithic kv block.  Zero-matmuls (+0 into an
accumulator, exact no-op) after each wave-A strip absorb the DMA-paced
idle so the HAM activity window never rethrottles the PE clock to 1.2
GHz mid-kernel.  Wave B (i-tiles 8-15) is tile-major: mask is fully
resident by then, and each tile's reciprocal + scale + store streams
out behind the PE instead of piling into a tail.
"""

import os
import sys
import types

sys.path.insert(0, "/opt/trn_rl_repo")

import numpy as np

import concourse.bacc as bacc
import concourse.tile as tile
from concourse import mybir
from concourse.bass_utils import run_bass_kernel_spmd


def _ensure_ntff_hook_importable():
    """bass_utils imports antenv.axon_hooks when BASS_TRACE is set; this
    image's antenv lacks that module.  Provide it (and register the real
    ctypes NTFF hook if available) so tracing works instead of crashing."""
    if "antenv.axon_hooks" in sys.modules:
        return
    try:
        import antenv
    except ImportError:
        return
    hooks = types.ModuleType("antenv.axon_hooks")
    hooks._hook = None
    hooks.set_axon_ntff_profile_hook = lambda h: setattr(hooks, "_hook", h)
    hooks.get_axon_ntff_profile_hook = lambda: hooks._hook
    sys.modules["antenv.axon_hooks"] = hooks
    antenv.axon_hooks = hooks
    try:
        from trn_agent_boot.trn_boot import _ntff_profile_via_ctypes

        hook = _ntff_profile_via_ctypes("/opt/axon/libaxon_pjrt.so")
        if hook is not None:
            hooks.set_axon_ntff_profile_hook(hook)
    except Exception:
        pass


_ensure_ntff_hook_importable()

P = 128
B = 8
L = 2048
D = 256
NT = L // P  # 16 strips / i-tiles per dim
NE = D + 2  # 258 = value cols + e col + zero pad (even moving width)

KVR_REC = D + NE  # per strip: k_s | [v|1|0]_s
HEAD_TOT = D + 4 * KVR_REC  # boot block: w | records for strips 0-3
KVR_TOT = (NT - 4) * KVR_REC  # records for strips 4-15

# strip groups: ev_s for a group becomes computable when its kv chunk lands
KV_GROUPS = ((0, 1, 2, 3), (4, 5, 6, 7), (8, 9, 10, 11), (12, 13, 14, 15))

N_UNPACK = 3  # strips NT-N_UNPACK..NT-1 arrive bit-packed, unpacked on DVE
UNPACK_S0 = NT - N_UNPACK

N_WARM_FREE = 8  # dep-free warm matmuls (N=512) at kernel start
N_WARM_KV = 1  # warm matmuls gated on the head DMA

LAST_RESULTS = None


def _build_nc():
    dt = mybir.dt
    nc = bacc.Bacc("TRN2", target_bir_lowering=False, debug=False, num_devices=B)

    maskt_d = nc.dram_tensor(
        "maskt", [P, UNPACK_S0 * L], dt.float8e4, kind="ExternalInput"
    ).ap()
    bits_d = nc.dram_tensor(
        "bits", [P, N_UNPACK * 256], dt.uint8, kind="ExternalInput"
    ).ap()
    head_d = nc.dram_tensor("head", [P, HEAD_TOT], dt.float16, kind="ExternalInput").ap()
    kvr_d = nc.dram_tensor("kvr", [P, KVR_TOT], dt.float16, kind="ExternalInput").ap()
    out_d = nc.dram_tensor("out", [P, NT * D], dt.float16, kind="ExternalOutput").ap()

    with tile.TileContext(nc) as tc:
        with (
            tc.tile_pool(name="const", bufs=1) as const_pool,
            tc.tile_pool(name="kv", bufs=1) as kv_pool,
            tc.tile_pool(name="small", bufs=1) as small_pool,
            tc.tile_pool(name="junk", bufs=2) as junk_pool,
            tc.tile_pool(name="outp", bufs=2) as out_pool,
            tc.tile_pool(name="rec", bufs=16) as rec_pool,
            tc.tile_pool(name="acc", bufs=8, space="PSUM") as acc_pool,
        ):
            # HAM warmup: dummy matmuls with no real dependencies to bring
            # the PE to full clock before data arrives.
            warm_mv = const_pool.tile([P, 512], dt.float16)
            nc.vector.memset(warm_mv[:], 0.0)
            warm_ps = acc_pool.tile([P, 512], dt.float32, tag="acc", name="warm")
            for _ in range(N_WARM_FREE):
                nc.tensor.matmul(
                    warm_ps[:], warm_mv[:, 0:P], warm_mv[:], start=True, stop=True
                )

            m8 = kv_pool.tile([P, UNPACK_S0 * L], dt.float8e4, tag="m8")
            bits = kv_pool.tile([P, N_UNPACK * 256], dt.uint8, tag="bits")
            head = kv_pool.tile([P, HEAD_TOT], dt.float16, tag="head")
            kvr = kv_pool.tile([P, KVR_TOT], dt.float16, tag="kvr")
            wrep = head[:, 0:D]

            def k_ap(s):
                if s < 4:
                    o = D + s * KVR_REC
                    return head[:, o : o + D]
                o = (s - 4) * KVR_REC
                return kvr[:, o : o + D]

            def vx_ap(s):
                if s < 4:
                    o = D + s * KVR_REC + D
                    return head[:, o : o + NE]
                o = (s - 4) * KVR_REC + D
                return kvr[:, o : o + NE]

            # ---- THE load ring (sync), in exact consumption order.
            def m_load(s_lo, s_hi):
                sl = slice(s_lo * L, (s_hi + 1) * L)
                nc.sync.dma_start(m8[:, sl], maskt_d[:, sl])

            def kv_load(gi):
                g = KV_GROUPS[gi]
                sl = slice((g[0] - 4) * KVR_REC, (g[-1] - 3) * KVR_REC)
                nc.sync.dma_start(kvr[:, sl], kvr_d[:, sl])

            nc.sync.dma_start(head[:], head_d[:])  # boot: w + strips 0-3
            m_load(0, 0)
            m_load(1, 1)
            nc.sync.dma_start(bits[:], bits_d[:])  # packed strips 13-15
            kv_load(1)  # s4-7
            m_load(2, 2)
            m_load(3, 4)
            kv_load(2)  # s8-11
            m_load(5, 6)
            m_load(7, 8)
            kv_load(3)  # s12-15
            m_load(9, 10)
            m_load(11, 12)

            # second warm burst, gated on the head DMA via its operands:
            # bridges the PE-idle gap to the first real matmul.
            for _ in range(N_WARM_KV):
                nc.tensor.matmul(
                    warm_ps[:], head[:, 0:P], head[:, 0:512], start=True, stop=True
                )

            # ---- prologue per kv group: sk = k.w ; e = exp(sk) ;
            # ev rows [e*v | e | 0] from the host-packed [v | 1 | 0].
            ub = kv_pool.tile([P, N_UNPACK * L], dt.uint8, tag="ub")
            m16 = kv_pool.tile([P, N_UNPACK * L], dt.float16, tag="m16")
            sk = small_pool.tile([P, NT], dt.float32, tag="sk")
            e_sb = small_pool.tile([P, NT], dt.float32, tag="e")
            ev = kv_pool.tile([P, NT * NE], dt.float16, tag="ev")
            ev3 = ev[:].rearrange("p (s n) -> p s n", n=NE)

            for gi, g in enumerate(KV_GROUPS):
                with tc.high_priority():
                    for s in g:
                        junk = junk_pool.tile([P, D], dt.float16, tag="junk")
                        nc.vector.scalar_tensor_tensor(
                            out=junk[:],
                            in0=k_ap(s),
                            scalar=1.0,
                            in1=wrep,
                            op0=mybir.AluOpType.mult,
                            op1=mybir.AluOpType.mult,
                            accum_out=sk[:, s : s + 1],
                        )
                        if gi == 0:
                            # per-strip exp + scale: strip s ready the moment
                            # its sk lands (these gate the stream head)
                            nc.scalar.activation(
                                e_sb[:, s : s + 1],
                                sk[:, s : s + 1],
                                mybir.ActivationFunctionType.Exp,
                            )
                            if s == 0:
                                nc.vector.tensor_scalar_mul(
                                    ev3[:, s, 0:NE], vx_ap(s), e_sb[:, s : s + 1]
                                )
                            else:
                                nc.scalar.mul(
                                    ev3[:, s, 0:NE], vx_ap(s), e_sb[:, s : s + 1]
                                )
                    if gi > 0:
                        cs = slice(g[0], g[-1] + 1)
                        nc.scalar.activation(
                            e_sb[:, cs], sk[:, cs], mybir.ActivationFunctionType.Exp
                        )
                        for s in g:
                            if s % 2 == 0 and gi < 3:
                                nc.vector.tensor_scalar_mul(
                                    ev3[:, s, 0:NE], vx_ap(s), e_sb[:, s : s + 1]
                                )
                            else:
                                nc.scalar.mul(
                                    ev3[:, s, 0:NE], vx_ap(s), e_sb[:, s : s + 1]
                                )
                # unpack batch for one strip per kv group: I1 and-masks on
                # DVE fill this group's kv-sem wait gap; the {0, 2^b} scale
                # cancels in the softmax division, and ACT Sign turns it
                # into clean {0,1} fp16 off the DVE
                if gi >= 1:
                    s_up = UNPACK_S0 + gi - 1
                    si = s_up - UNPACK_S0
                    srcb = bits[:, si * 256 : (si + 1) * 256]
                    with tc.tile_wait_until(0.0100 + 0.0012 * si):
                        for bpl in range(8):
                            nc.vector.tensor_scalar(
                                out=ub[:, si * L + 256 * bpl : si * L + 256 * (bpl + 1)],
                                in0=srcb,
                                scalar1=1 << bpl,
                                scalar2=None,
                                op0=mybir.AluOpType.bitwise_and,
                            )
                        nc.scalar.sign(
                            m16[:, si * L : (si + 1) * L],
                            ub[:, si * L : (si + 1) * L],
                        )

            def mask_tile(s, t):
                if s >= UNPACK_S0:
                    o = (s - UNPACK_S0) * L
                    return m16[:, o + t * P : o + (t + 1) * P]
                return m8[:, s * L + t * P : s * L + (t + 1) * P]

            def epilogue(acc, t, outb, ti):
                rec = rec_pool.tile([P, 1], dt.float32, tag="rec", name=f"r{t}")
                nc.vector.reciprocal(rec[:], acc[:, D : D + 1])
                ob = outb[:, ti * D : (ti + 1) * D]
                if ti % 2 == 0:
                    nc.scalar.mul(ob, acc[:, 0:D], rec[:])
                else:
                    nc.vector.tensor_scalar_mul(ob, acc[:, 0:D], rec[:])

            # ---- wave A: i-tiles 0-7, strip-major (matches DMA arrival).
            # Wave A is DMA-paced; zero-matmuls (+0 accumulate, exact no-op)
            # after each strip absorb the idle and keep the HAM window busy.
            accs = [
                acc_pool.tile([P, NE], dt.float32, tag="acc", name=f"acc{t}")
                for t in range(8)
            ]
            outbA = out_pool.tile([P, 8 * D], dt.float16, tag="outb", name="outbA")
            for s in range(NT):
                mov = ev3[:, s, 0:NE]
                for t in range(8):
                    nc.tensor.matmul(
                        accs[t][:],
                        mask_tile(s, t),
                        mov,
                        start=(s == 0),
                        stop=(s == NT - 1),
                    )
                if 1 <= s <= 7:
                    for _ in range(2 if s <= 3 else 1):
                        nc.tensor.matmul(
                            accs[7][:],
                            warm_mv[:, 0:P],
                            warm_mv[:, 0:NE],
                            start=False,
                            stop=False,
                        )
            for t in range(8):
                epilogue(accs[t], t, outbA, t)
                if t == 3:
                    nc.sync.dma_start(out_d[:, 0 : 4 * D], outbA[:, 0 : 4 * D])
                elif t == 7:
                    nc.sync.dma_start(out_d[:, 4 * D : 8 * D], outbA[:, 4 * D : 8 * D])

            # ---- wave B: i-tiles 8-15, tile-major (mask fully resident);
            # each tile's epilogue + store streams behind the PE.
            outbB = out_pool.tile([P, 8 * D], dt.float16, tag="outb", name="outbB")
            for t in range(8, NT):
                ti = t - 8
                accB = acc_pool.tile([P, NE], dt.float32, tag="acc", name=f"acc{t}")
                for s in range(NT):
                    nc.tensor.matmul(
                        accB[:],
                        mask_tile(s, t),
                        ev3[:, s, 0:NE],
                        start=(s == 0),
                        stop=(s == NT - 1),
                    )
                epilogue(accB, t, outbB, ti)
                # stores: pairs for tiles 8-13, singles for 14/15 so the
                # final store (and its HBM write receipt) is small and early
                if t in (9, 11, 13):
                    nc.sync.dma_start(
                        out_d[:, (t - 1) * D : (t + 1) * D],
                        outbB[:, (ti - 1) * D : (ti + 1) * D],
                    )
                elif t >= 14:
                    nc.sync.dma_start(
                        out_d[:, t * D : (t + 1) * D],
                        outbB[:, ti * D : (ti + 1) * D],
                    )

    nc.compile()
    return nc


def kernel(query, key, value, mask, w_align):
    global LAST_RESULTS
    key = np.asarray(key, dtype=np.float32)
    value = np.asarray(value, dtype=np.float32)
    mask = np.asarray(mask)
    w_align = np.asarray(w_align, dtype=np.float32)

    import ml_dtypes

    nc = _build_nc()
    in_maps = []
    for b in range(B):
        # maskt[p, s*L + t*128+c] = mask[b][i=128t+c, j=128s+p], as fp8e4
        # bytes: 0x38 is 1.0 in fp8_e4m3 (bias 7)
        mu = mask[b].astype(np.uint8)  # [i, j]
        mt = (
            (mu * np.uint8(0x38))
            .reshape(NT, P, NT, P)  # [t, c, s, p]
            .transpose(3, 2, 0, 1)  # [p, s, t, c]
            .reshape(P, NT * L)[:, : UNPACK_S0 * L]
        )
        # bit-packed tail strips: bit b of bits[p, si*256 + w] is
        # mask[i = 256b + w, j = s*128 + p]
        bitsb = np.zeros((P, N_UNPACK * 256), dtype=np.uint8)
        sh = np.arange(8, dtype=np.uint8)[:, None, None]
        for s in range(UNPACK_S0, NT):
            blk = mu[:, s * P : (s + 1) * P]  # [2048 i, 128 p]
            packed = (blk.reshape(8, 256, P) << sh).sum(0).astype(np.uint8)
            bitsb[:, (s - UNPACK_S0) * 256 : (s - UNPACK_S0 + 1) * 256] = packed.T
        kb = key[b].reshape(NT, P, D).transpose(1, 0, 2)  # [p, s, d]
        vb = value[b].reshape(NT, P, D).transpose(1, 0, 2)
        headb = np.zeros((P, HEAD_TOT), dtype=np.float16)
        headb[:, 0:D] = w_align[None, :]
        for s in range(4):
            o = D + s * KVR_REC
            headb[:, o : o + D] = kb[:, s]
            headb[:, o + D : o + D + D] = vb[:, s]
            headb[:, o + D + D] = 1.0
        kvrb = np.zeros((P, KVR_TOT), dtype=np.float16)
        for s in range(4, NT):
            o = (s - 4) * KVR_REC
            kvrb[:, o : o + D] = kb[:, s]
            kvrb[:, o + D : o + D + D] = vb[:, s]
            kvrb[:, o + D + D] = 1.0  # ones col -> e in moving col 256
            # col 257 stays 0 (pad for even moving width)
        in_maps.append(
            {
                "maskt": np.ascontiguousarray(mt).view(ml_dtypes.float8_e4m3),
                "bits": bitsb,
                "head": headb,
                "kvr": kvrb,
            }
        )
    try:
        res = run_bass_kernel_spmd(nc, in_maps, core_ids=list(range(B)))
    except Exception:
        # e.g. trace requested but profiling unavailable -- retry untraced
        os.environ["BASS_NEVER_TRACE"] = "1"
        res = run_bass_kernel_spmd(nc, in_maps, core_ids=list(range(B)))
    LAST_RESULTS = res
    out = np.empty((B, L, D), dtype=np.float32)
    for b in range(B):
        ob = res.results[b]["out"].astype(np.float32)  # [p, t*D]
        out[b] = ob.reshape(P, NT, D).transpose(1, 0, 2).reshape(L, D)
    return out
